# revision 1
# baseline (speedup 1.0000x reference)
"""Trainium2 Bass kernel for nn_AdaptiveFunctionBlock (gnn_message_passing).

Row-shards the N=4096 dimension across 8 NeuronCores.  Per core (512 rows):
  step s: L = ((Xa@W1)@W2)@(Xa@W3)^T in f32, per-row top-p (nucleus) selection
  done exactly (segment top-8 candidates -> sorted top-T -> f32 sequential
  cumsum threshold, reference-faithful tie handling via a column correction),
  softmax -> A, then bf16 matmuls  Xa' = A@Xa,  Xf' = recipD*(S@Xf + Xf_loc),
  Z += Xf'@U1_s + Xa'@U2_s,  with an AllGather of (Xf'|Xa') between steps.
  Step-2 logits use associativity: K2 = A1@(X@W3), Q2' = A1@((X@W1)@W2), so the
  f32 work stays O(N^2*64).  Finally LayerNorm(X + Z)*gamma + beta.
"""

import numpy as np

N = 4096
D = 1024
DA = 64
NCORES = 8
NLOC = N // NCORES          # 512
NT = NLOC // 128            # 4 row-tiles per core
NJC = N // 128              # 32 column chunks
P_TOP = 0.9
LN_EPS = 1e-5
T1, T2 = 16, 96             # top-p extraction depth per step
NSEG = 32
SEGW = N // NSEG            # 128
NEGINF = -3.0e38
POSINF = 3.0e38

_CACHE = {}


def _build(phase=5, sub=9, iters=1):
    import concourse.bass as bass
    import concourse.mybir as mybir
    from concourse import bacc, tile

    dt = mybir.dt
    f32 = dt.float32
    bf16 = dt.bfloat16
    i32 = dt.int32
    u32 = dt.uint32
    Act = mybir.ActivationFunctionType
    Alu = mybir.AluOpType
    PSUM = bass.MemorySpace.PSUM
    DRAM = bass.MemorySpace.DRAM

    nc = bacc.Bacc(num_devices=NCORES)

    # ---------------- I/O ----------------
    X_d = nc.dram_tensor("X", [N, D], f32, kind="ExternalInput")
    Xloc_d = nc.dram_tensor("Xloc", [NLOC, D], f32, kind="ExternalInput")
    S_d = nc.dram_tensor("S", [NLOC, N], f32, kind="ExternalInput")
    W1_d = nc.dram_tensor("W1", [D, DA], f32, kind="ExternalInput")
    W2_d = nc.dram_tensor("W2", [DA, DA], f32, kind="ExternalInput")
    W3_d = nc.dram_tensor("W3", [D, DA], f32, kind="ExternalInput")
    U_d = [
        nc.dram_tensor("U1_0", [D, D], f32, kind="ExternalInput"),
        nc.dram_tensor("U2_0", [D, D], f32, kind="ExternalInput"),
        nc.dram_tensor("U1_1", [D, D], f32, kind="ExternalInput"),
        nc.dram_tensor("U2_1", [D, D], f32, kind="ExternalInput"),
    ]
    gamma_d = nc.dram_tensor("gamma", [1, D], f32, kind="ExternalInput")
    beta_d = nc.dram_tensor("beta", [1, D], f32, kind="ExternalInput")
    out_d = nc.dram_tensor("out", [NLOC, D], f32, kind="ExternalOutput")

    # ---------------- inline constants ----------------
    ident_c = nc.inline_tensor(np.eye(128, dtype=np.float32), name="ident_c")
    ones1_c = nc.inline_tensor(np.ones((1, 128), dtype=np.float32), name="ones1_c")

    rg = [list(range(NCORES))]
    Tmax = max(T1, T2)

    with tile.TileContext(nc) as tc:
        with (
            tc.tile_pool(name="const", bufs=1) as cpool,
            tc.tile_pool(name="res", bufs=1) as rpool,
            tc.tile_pool(name="big", bufs=1) as bpool,
            tc.tile_pool(name="stream", bufs=1) as stpool,
            tc.tile_pool(name="small", bufs=1) as spool,
            tc.tile_pool(name="psum", bufs=1, space=PSUM) as ppool,
            tc.tile_pool(name="dram", bufs=1, space=DRAM) as dpool,
        ):
            # every PSUM user goes through one tag: 8 x [128,512] banks
            def ps_tile(name):
                return ppool.tile([128, 512], f32, tag="pb", bufs=8, name=name)

            def ps_tile_b(name):
                return ppool.tile([128, 512], bf16, tag="pb", bufs=8, name=name)

            # ---------- SBUF consts ----------
            identf_s = cpool.tile([128, 128], f32, name="identf_s")
            nc.sync.dma_start(identf_s[:], ident_c[:])
            identb_s = cpool.tile([128, 128], bf16, name="identb_s")
            nc.vector.tensor_copy(identb_s[:], identf_s[:])
            ones1_s = cpool.tile([1, 128], f32, name="ones1_s")
            nc.sync.dma_start(ones1_s[:], ones1_c[:])
            zerosT_s = cpool.tile([128, Tmax], f32, name="zerosT_s")
            nc.vector.memset(zerosT_s[:], 0.0)

            # ---------- HBM scratch ----------
            Xb_hbm = dpool.tile([NJC, 128, D], bf16, name="Xb_hbm")
            STb_hbm = dpool.tile([NJC, 128, NLOC], bf16, name="STb_hbm")
            ATb_hbm = dpool.tile([NJC, 128, NLOC], bf16, name="ATb_hbm")

            # ---------- residents ----------
            recipD_s = rpool.tile([128, NT], f32, name="recipD_s")
            Z_s = rpool.tile([128, NT, D], f32, name="Z_s")
            K1P1rm_s = rpool.tile([128, NJC, 128], f32, name="K1P1rm_s")
            Xf_loc = rpool.tile([128, NT, D], bf16, name="Xf_loc")
            Xa_loc = rpool.tile([128, NT, D], bf16, name="Xa_loc")

            for rep_ in range(iters):
                # ================= PROLOGUE =================
                ag_kp1_in = dpool.tile([128, NLOC], f32, name="ag_kp1_in")
                ag_kp1_out = dpool.tile(
                    [NCORES, 128, NLOC], f32, addr_space="Shared", name="ag_kp1_out"
                )
                ag_kp2_in = dpool.tile([128, NLOC], f32, name="ag_kp2_in")
                ag_kp2_out = dpool.tile(
                    [NCORES, 128, NLOC], f32, addr_space="Shared", name="ag_kp2_out"
                )
                ag_x_in = dpool.tile([NLOC, 2 * D], bf16, name="ag_x_in")
                ag_x_out = dpool.tile(
                    [NCORES, NLOC, 2 * D], bf16, addr_space="Shared", name="ag_x_out"
                )
                # Xloc load + bf16 + transpose X^T (for K1/Q1 matmuls)
                Xloc_b = bpool.tile([128, NT, D], bf16, tag="XfT", bufs=2, name="Xloc_b")
                XT_s = bpool.tile([128, 8, NLOC], f32, tag="E", bufs=1, name="XT_s")
                for it in range(NT):
                    xl = stpool.tile([128, D], f32, tag="x4k", bufs=2, name="xl")
                    nc.gpsimd.dma_start(xl[:], Xloc_d[it * 128 : (it + 1) * 128, :])
                    nc.vector.tensor_copy(Xloc_b[:, it, :], xl[:])
                    for dq in range(2):  # 2 groups of 4 transpose blocks
                        tp = ps_tile(f"tpx{it}{dq}")
                        for b in range(4):
                            dc = dq * 4 + b
                            nc.tensor.transpose(
                                tp[:, b * 128 : (b + 1) * 128],
                                xl[:, dc * 128 : (dc + 1) * 128],
                                identf_s[:],
                            )
                        for b in range(4):
                            dc = dq * 4 + b
                            nc.scalar.activation(
                                XT_s[:, dc, it * 128 : (it + 1) * 128],
                                tp[:, b * 128 : (b + 1) * 128],
                                Act.Copy,
                            )

                # W31 = [W3|W1] chunks
                W31_s = stpool.tile([128, 8, 128], f32, tag="x4k", bufs=2, name="W31_s")
                nc.sync.dma_start(
                    W31_s[:, :, 0:DA], W3_d[:].rearrange("(c p) a -> p c a", p=128)
                )
                nc.sync.dma_start(
                    W31_s[:, :, DA:128], W1_d[:].rearrange("(c p) a -> p c a", p=128)
                )
                W2_s = spool.tile([DA, DA], f32, tag="W2", bufs=1, name="W2_s")
                nc.sync.dma_start(W2_s[:], W2_d[:])

                # K1^T|Q1^T = W31^T @ X^T   (f32)
                kqp = ps_tile("kqp")
                for dc in range(8):
                    nc.tensor.matmul(
                        kqp[:],
                        W31_s[:, dc, :],
                        XT_s[:, dc, :],
                        start=(dc == 0),
                        stop=(dc == 7),
                    )
                KP1_s = rpool.tile([128, NLOC], f32, name="KP1_s")
                nc.scalar.activation(KP1_s[0:64, :], kqp[0:64, :], Act.Copy)
                q1t = spool.tile([64, NLOC], f32, tag="q1t", bufs=1, name="q1t")
                nc.scalar.activation(q1t[:], kqp[64:128, :], Act.Copy)
                p1p = ps_tile("p1p")
                nc.tensor.matmul(p1p[0:64, :], W2_s[:], q1t[:], start=True, stop=True)
                nc.scalar.activation(KP1_s[64:128, :], p1p[0:64, :], Act.Copy)

                nc.sync.dma_start(ag_kp1_in[:], KP1_s[:])
                nc.gpsimd.collective_compute(
                    "AllGather",
                    Alu.bypass,
                    replica_groups=rg,
                    ins=[ag_kp1_in[:].opt()],
                    outs=[ag_kp1_out[:].opt()],
                )

                # X -> bf16 HBM copy (streamed; overlaps AG_kp1)
                for jc in range(NJC if (phase != 1 or sub >= 1) else 0):
                    xin = stpool.tile([128, D], f32, tag="x4k", bufs=2, name="xin")
                    nc.gpsimd.dma_start(xin[:], X_d[jc * 128 : (jc + 1) * 128, :])
                    xbf = stpool.tile([128, D], bf16, tag="x2k", bufs=2, name="xbf")
                    nc.vector.tensor_copy(xbf[:], xin[:])
                    nc.sync.dma_start(Xb_hbm[jc, :, :], xbf[:])

                # S prep: recipD = 1/(rowsum+1); S^T bf16 -> HBM
                for it in range(NT if (phase != 1 or sub >= 2) else 0):
                    st = bpool.tile([128, N], f32, tag="L", bufs=1, name="st")
                    nc.sync.dma_start(st[:], S_d[it * 128 : (it + 1) * 128, :])
                    rs = spool.tile([128, 1], f32, tag="rs", bufs=2, name="rs")
                    nc.vector.tensor_reduce(rs[:], st[:], mybir.AxisListType.X, Alu.add)
                    rsp = spool.tile([128, 1], f32, tag="rsp", bufs=2, name="rsp")
                    nc.vector.tensor_scalar_add(rsp[:], rs[:], 1.0)
                    nc.vector.reciprocal(recipD_s[:, it : it + 1], rsp[:])
                    sb = stpool.tile([128, N], bf16, tag="ATb", bufs=1, name="sb")
                    nc.vector.tensor_copy(sb[:], st[:])
                    stg = stpool.tile(
                        [128, NJC, 128], bf16, tag="ATb", bufs=1, name="stg"
                    )
                    for jq in range(8):  # 32 blocks in groups of 4
                        tp2 = ps_tile_b(f"tps{it}{jq}")
                        for b in range(4):
                            jc = jq * 4 + b
                            nc.tensor.transpose(
                                tp2[:, b * 128 : (b + 1) * 128],
                                sb[:, jc * 128 : (jc + 1) * 128],
                                identb_s[:],
                            )
                        for b in range(4):
                            jc = jq * 4 + b
                            nc.scalar.activation(
                                stg[:, jc, :], tp2[:, b * 128 : (b + 1) * 128], Act.Copy
                            )
                    nc.sync.dma_start(
                        STb_hbm[:, :, it * 128 : (it + 1) * 128].rearrange(
                            "c p i -> p c i"
                        ),
                        stg[:],
                    )

                # K1^T full  (f32, parts 0:64)
                K1T_s = bpool.tile([128, N // 2], f32, tag="KT", bufs=1, name="K1T_s")
                for r in range(NCORES):
                    ph, off = (0, r) if r < 4 else (64, r - 4)
                    nc.sync.dma_start(
                        K1T_s[ph : ph + 64, off * NLOC : (off + 1) * NLOC],
                        ag_kp1_out[r, 0:64, :],
                    )
                # K1P1 row-major via transposes of AG blocks
                for rq in range(8 if (phase != 1 or sub >= 3) else 0):  # 32 blocks in groups of 4
                    tp3 = ps_tile(f"tpk{rq}")
                    blks = []
                    for b in range(4):
                        g = rq * 4 + b
                        r_, ib = g // NT, g % NT
                        blk = stpool.tile(
                            [128, 128], f32, tag="kpblk", bufs=2, name=f"blk{g}"
                        )
                        nc.sync.dma_start(
                            blk[:], ag_kp1_out[r_, :, ib * 128 : (ib + 1) * 128]
                        )
                        nc.tensor.transpose(
                            tp3[:, b * 128 : (b + 1) * 128], blk[:], identf_s[:]
                        )
                    for b in range(4):
                        g = rq * 4 + b
                        nc.scalar.activation(
                            K1P1rm_s[:, g, :], tp3[:, b * 128 : (b + 1) * 128], Act.Copy
                        )

                KP2_s = rpool.tile([128, NLOC], f32, name="KP2_s")
                kp2ps = None

                # ================= STEPS =================
                nsteps = 0 if phase < 2 else (1 if phase < 4 else 2)
                for s in range(nsteps):
                    T = T1 if s == 0 else T2
                    if s == 0:
                        KPl, KTf = KP1_s, K1T_s
                    else:
                        K2T_s = bpool.tile(
                            [128, N // 2], f32, tag="KT", bufs=1, name="K2T_s"
                        )
                        for r in range(NCORES):
                            ph, off = (0, r) if r < 4 else (r - 4, r - 4)
                            ph = 0 if r < 4 else 64
                            off = r if r < 4 else r - 4
                            nc.sync.dma_start(
                                K2T_s[ph : ph + 64, off * NLOC : (off + 1) * NLOC],
                                ag_kp2_out[r, 0:64, :],
                            )
                        KPl, KTf = KP2_s, K2T_s

                    # P^T duplicated in both partition halves (matmul lhsT must
                    # share base partition with the K^T rhs half it pairs with)
                    PT_s = spool.tile([128, NLOC], f32, tag="PT", bufs=1, name=f"PT{s}")
                    nc.sync.dma_start(PT_s[0:64, :], KPl[64:128, :])
                    nc.sync.dma_start(PT_s[64:128, :], KPl[64:128, :])

                    # per-row-tile: logits -> stats -> A
                    for it in range(NT):
                        L = bpool.tile([128, N], f32, tag="L", bufs=1, name="L")
                        for jc8 in range(8):
                            lp = ps_tile(f"lp{it}{jc8}")
                            kph = 0 if jc8 < 4 else 64
                            kpo = jc8 if jc8 < 4 else jc8 - 4
                            nc.tensor.matmul(
                                lp[:],
                                PT_s[kph : kph + 64, it * 128 : (it + 1) * 128],
                                KTf[kph : kph + 64, kpo * 512 : (kpo + 1) * 512],
                                start=True,
                                stop=True,
                            )
                            nc.scalar.activation(
                                L[:, jc8 * 512 : (jc8 + 1) * 512], lp[:], Act.Copy
                            )

                        if sub < 1:
                            continue
                        # --- candidates: per-segment top-8
                        cand = spool.tile([128, 256], f32, tag="cand", bufs=2, name="cand")
                        for sg in range(NSEG):
                            nc.vector.max(
                                cand[:, sg * 8 : sg * 8 + 8],
                                L[:, sg * SEGW : (sg + 1) * SEGW],
                            )
                        if s == 1:
                            cand_copy = spool.tile(
                                [128, 256], f32, tag="candc", bufs=2, name="cand_copy"
                            )
                            nc.vector.tensor_copy(cand_copy[:], cand[:])

                        if sub < 2:
                            continue
                        # --- extract sorted top-T (destroys cand)
                        V = spool.tile([128, Tmax], f32, tag="V", bufs=2, name="V")
                        for rnd in range(T // 8):
                            nc.vector.max(V[:, rnd * 8 : rnd * 8 + 8], cand[:])
                            if rnd < T // 8 - 1:
                                nc.vector.match_replace(
                                    cand[:], V[:, rnd * 8 : rnd * 8 + 8], cand[:], NEGINF
                                )

                        if sub < 3:
                            continue
                        negm = spool.tile([128, 1], f32, tag="negm", bufs=2, name="negm")
                        nc.vector.tensor_scalar_mul(negm[:], V[:, 0:1], -1.0)

                        E = bpool.tile([128, N], f32, tag="E", bufs=1, name="E")
                        Zrow = spool.tile([128, 1], f32, tag="Zrow", bufs=2, name="Zrow")
                        nc.scalar.activation(
                            E[:], L[:], Act.Exp, bias=negm[:], accum_out=Zrow[:]
                        )
                        EV = spool.tile([128, Tmax], f32, tag="EV", bufs=2, name="EV")
                        nc.scalar.activation(EV[:, 0:T], V[:, 0:T], Act.Exp, bias=negm[:])
                        cs = spool.tile([128, Tmax], f32, tag="cs", bufs=2, name="cs")
                        nc.vector.tensor_tensor_scan(
                            cs[:, 0:T], EV[:, 0:T], zerosT_s[:, 0:T], 0.0, Alu.add, Alu.add
                        )
                        thr = spool.tile([128, 1], f32, tag="thr", bufs=2, name="thr")
                        nc.vector.tensor_scalar_mul(thr[:], Zrow[:], P_TOP)
                        kept = spool.tile([128, Tmax], f32, tag="kept", bufs=2, name="kept")
                        nc.vector.scalar_tensor_tensor(
                            kept[:, 0:T], cs[:, 0:T], thr[:], EV[:, 0:T],
                            Alu.subtract, Alu.is_lt,
                        )
                        scr1 = spool.tile([128, Tmax], f32, tag="scr1", bufs=1, name="scr1")
                        Drow = spool.tile([128, 1], f32, tag="Drow", bufs=2, name="Drow")
                        nc.vector.tensor_tensor(
                            scr1[:, 0:T], EV[:, 0:T], kept[:, 0:T], Alu.mult
                        )
                        nc.vector.tensor_reduce(
                            Drow[:], scr1[:, 0:T], mybir.AxisListType.X, Alu.add
                        )
                        nki = spool.tile([128, Tmax], f32, tag="nki", bufs=1, name="nki")
                        nc.vector.tensor_scalar(
                            nki[:, 0:T], kept[:, 0:T], 0.5, POSINF, Alu.is_lt, Alu.mult
                        )
                        scr2 = spool.tile([128, Tmax], f32, tag="scr2", bufs=1, name="scr2")
                        tau = spool.tile([128, 1], f32, tag="tau", bufs=2, name="tau")
                        nc.vector.tensor_tensor(
                            scr2[:, 0:T], nki[:, 0:T], V[:, 0:T], Alu.add
                        )
                        nc.vector.tensor_reduce(
                            tau[:], scr2[:, 0:T], mybir.AxisListType.X, Alu.min
                        )
                        recD = spool.tile([128, 1], f32, tag="recD", bufs=2, name="recD")
                        nc.vector.reciprocal(recD[:], Drow[:])
                        etau = spool.tile([128, 1], f32, tag="etau", bufs=2, name="etau")
                        nc.scalar.activation(etau[:], tau[:], Act.Exp, bias=negm[:])

                        if s == 1:
                            scrT = spool.tile(
                                [128, Tmax], f32, tag="scrT", bufs=1, name="scrT"
                            )
                            rr = spool.tile([128, 1], f32, tag="rr", bufs=2, name="rr")
                            nc.vector.scalar_tensor_tensor(
                                scrT[:, 0:T], V[:, 0:T], tau[:], kept[:, 0:T],
                                Alu.is_equal, Alu.mult,
                            )
                            nc.vector.tensor_reduce(
                                rr[:], scrT[:, 0:T], mybir.AxisListType.X, Alu.add
                            )
                            scr256 = spool.tile(
                                [128, 256], f32, tag="scr256", bufs=1, name="scr256"
                            )
                            ceq = spool.tile([128, 1], f32, tag="ceq", bufs=2, name="ceq")
                            nc.vector.tensor_scalar(
                                scr256[:], cand_copy[:], tau[:], None, Alu.is_equal
                            )
                            nc.vector.tensor_reduce(
                                ceq[:], scr256[:], mybir.AxisListType.X, Alu.add
                            )
                            # w2 = (ceq - r)/ceq * etau * recD  (spread over all ties)
                            wv = spool.tile([128, 1], f32, tag="wv", bufs=2, name="wv")
                            nc.vector.tensor_tensor(wv[:], ceq[:], rr[:], Alu.subtract)
                            nc.vector.tensor_tensor(wv[:], wv[:], etau[:], Alu.mult)
                            nc.vector.tensor_tensor(wv[:], wv[:], recD[:], Alu.mult)
                            rceq = spool.tile([128, 1], f32, tag="rceq", bufs=2, name="rceq")
                            nc.vector.reciprocal(rceq[:], ceq[:])
                            nc.vector.tensor_tensor(wv[:], wv[:], rceq[:], Alu.mult)

                        if sub < 4:
                            continue
                        # --- A materialization (E in-place): E := (E >= etau) * E
                        nc.vector.scalar_tensor_tensor(
                            E[:], E[:], etau[:], E[:], Alu.is_ge, Alu.mult
                        )
                        if s == 0:
                            Af = bpool.tile([128, N], f32, tag="Af", bufs=1, name="Af")
                            nc.vector.tensor_scalar_mul(Af[:], E[:], recD[:])
                            ATf = bpool.tile(
                                [128, NJC, 128], f32, tag="ATf", bufs=1, name="ATf"
                            )
                            for jq in range(8):
                                tp4 = ps_tile(f"tpa{it}{jq}")
                                for b in range(4):
                                    jc = jq * 4 + b
                                    nc.tensor.transpose(
                                        tp4[:, b * 128 : (b + 1) * 128],
                                        Af[:, jc * 128 : (jc + 1) * 128],
                                        identf_s[:],
                                    )
                                for b in range(4):
                                    jc = jq * 4 + b
                                    nc.scalar.activation(
                                        ATf[:, jc, :],
                                        tp4[:, b * 128 : (b + 1) * 128],
                                        Act.Copy,
                                    )
                            ATb = stpool.tile(
                                [128, NJC, 128], bf16, tag="ATb", bufs=1, name="ATb"
                            )
                            nc.vector.tensor_copy(ATb[:], ATf[:])
                            nc.sync.dma_start(
                                ATb_hbm[:, :, it * 128 : (it + 1) * 128].rearrange(
                                    "c p i -> p c i"
                                ),
                                ATb[:],
                            )
                            # K2P2^T accumulation (held psum bank across it loop)
                            if it == 0:
                                kp2ps = ps_tile("kp2ps")
                            for jc in range(NJC):
                                nc.tensor.matmul(
                                    kp2ps[:, it * 128 : (it + 1) * 128],
                                    K1P1rm_s[:, jc, :],
                                    ATf[:, jc, :],
                                    start=(jc == 0),
                                    stop=(jc == NJC - 1),
                                )
                        else:
                            Ab = bpool.tile([128, N], bf16, tag="Af", bufs=1, name="Ab")
                            nc.vector.tensor_scalar_mul(Ab[:], E[:], recD[:])
                            # tie-drop correction: A[:, L==tau] -= w2 (rows identical)
                            eqw = bpool.tile([128, N], f32, tag="ATf", bufs=1, name="eqw")
                            nc.vector.tensor_scalar(
                                eqw[:], L[:], tau[:], wv[:], Alu.is_equal, Alu.mult
                            )
                            nc.vector.tensor_tensor(Ab[:], Ab[:], eqw[:], Alu.subtract)
                            ATb2 = stpool.tile(
                                [128, NJC, 128], bf16, tag="ATb", bufs=1, name="ATb2"
                            )
                            for jq in range(8):
                                tp5 = ps_tile_b(f"tpb{it}{jq}")
                                for b in range(4):
                                    jc = jq * 4 + b
                                    nc.tensor.transpose(
                                        tp5[:, b * 128 : (b + 1) * 128],
                                        Ab[:, jc * 128 : (jc + 1) * 128],
                                        identb_s[:],
                                    )
                                for b in range(4):
                                    jc = jq * 4 + b
                                    nc.scalar.activation(
                                        ATb2[:, jc, :],
                                        tp5[:, b * 128 : (b + 1) * 128],
                                        Act.Copy,
                                    )
                            nc.sync.dma_start(
                                ATb_hbm[:, :, it * 128 : (it + 1) * 128].rearrange(
                                    "c p i -> p c i"
                                ),
                                ATb2[:],
                            )

                    if s == 0 and kp2ps is not None:
                        nc.scalar.activation(KP2_s[:], kp2ps[:], Act.Copy)
                        nc.sync.dma_start(ag_kp2_in[:], KP2_s[:])
                        nc.gpsimd.collective_compute(
                            "AllGather",
                            Alu.bypass,
                            replica_groups=rg,
                            ins=[ag_kp2_in[:].opt()],
                            outs=[ag_kp2_out[:].opt()],
                        )

                    if phase < 3:
                        continue

                    # ---------- big matmuls
                    xdiag = Xloc_b if s == 0 else Xf_loc
                    for dh in range(2):
                        xfp = [None] * NT
                        xap = [None] * NT
                        for it in range(NT):
                            xfp[it] = ps_tile(f"xfp{s}{dh}{it}")
                            xap[it] = ps_tile(f"xap{s}{dh}{it}")
                        for jc in range(NJC):
                            stch = stpool.tile(
                                [128, NLOC], bf16, tag="stch", bufs=2, name="stch"
                            )
                            nc.sync.dma_start(stch[:], STb_hbm[jc, :, :])
                            atch = stpool.tile(
                                [128, NLOC], bf16, tag="atch", bufs=2, name="atch"
                            )
                            nc.sync.dma_start(atch[:], ATb_hbm[jc, :, :])
                            if s == 0:
                                xch_f = stpool.tile(
                                    [128, 512], bf16, tag="xch", bufs=2, name="xchf"
                                )
                                nc.sync.dma_start(
                                    xch_f[:], Xb_hbm[jc, :, dh * 512 : (dh + 1) * 512]
                                )
                                xch_a = xch_f
                            else:
                                r_, blk_ = jc // NT, jc % NT
                                xch_f = stpool.tile(
                                    [128, 512], bf16, tag="xch", bufs=2, name="xchf"
                                )
                                nc.sync.dma_start(
                                    xch_f[:],
                                    ag_x_out[
                                        r_,
                                        blk_ * 128 : (blk_ + 1) * 128,
                                        dh * 512 : (dh + 1) * 512,
                                    ],
                                )
                                xch_a = stpool.tile(
                                    [128, 512], bf16, tag="xcha", bufs=2, name="xcha"
                                )
                                nc.sync.dma_start(
                                    xch_a[:],
                                    ag_x_out[
                                        r_,
                                        blk_ * 128 : (blk_ + 1) * 128,
                                        D + dh * 512 : D + (dh + 1) * 512,
                                    ],
                                )
                            for it in range(NT):
                                nc.tensor.matmul(
                                    xfp[it][:],
                                    stch[:, it * 128 : (it + 1) * 128],
                                    xch_f[:],
                                    start=(jc == 0),
                                    stop=(jc == NJC - 1),
                                )
                                nc.tensor.matmul(
                                    xap[it][:],
                                    atch[:, it * 128 : (it + 1) * 128],
                                    xch_a[:],
                                    start=(jc == 0),
                                    stop=(jc == NJC - 1),
                                )
                        for it in range(NT):
                            # Xf' = recipD*(psum + xdiag)
                            tsum = stpool.tile(
                                [128, 512], f32, tag="tsum", bufs=1, name="tsum"
                            )
                            nc.vector.tensor_tensor(
                                tsum[:],
                                xfp[it][:],
                                xdiag[:, it, dh * 512 : (dh + 1) * 512],
                                Alu.add,
                            )
                            nc.vector.tensor_scalar_mul(
                                Xf_loc[:, it, dh * 512 : (dh + 1) * 512],
                                tsum[:],
                                recipD_s[:, it : it + 1],
                            )
                            nc.vector.tensor_copy(
                                Xa_loc[:, it, dh * 512 : (dh + 1) * 512], xap[it][:]
                            )

                    if s == 0:
                        for it in range(NT):
                            nc.sync.dma_start(
                                ag_x_in[it * 128 : (it + 1) * 128, 0:D], Xf_loc[:, it, :]
                            )
                            nc.sync.dma_start(
                                ag_x_in[it * 128 : (it + 1) * 128, D : 2 * D],
                                Xa_loc[:, it, :],
                            )
                        nc.gpsimd.collective_compute(
                            "AllGather",
                            Alu.bypass,
                            replica_groups=rg,
                            ins=[ag_x_in[:].opt()],
                            outs=[ag_x_out[:].opt()],
                        )

                    # ---------- Xf^T / Xa^T
                    XfT_s = bpool.tile(
                        [128, 8, NLOC], bf16, tag="XfT", bufs=2, name=f"XfT{s}"
                    )
                    XaT_s = bpool.tile(
                        [128, 8, NLOC], bf16, tag="XfT", bufs=2, name=f"XaT{s}"
                    )
                    for src, dst in ((Xf_loc, XfT_s), (Xa_loc, XaT_s)):
                        for dc in range(8):
                            tpt = ps_tile_b(f"tpt{s}{dc}")
                            for it in range(NT):
                                nc.tensor.transpose(
                                    tpt[:, it * 128 : (it + 1) * 128],
                                    src[:, it, dc * 128 : (dc + 1) * 128],
                                    identb_s[:],
                                )
                            nc.scalar.activation(dst[:, dc, :], tpt[:], Act.Copy)

                    # ---------- U products: Z += Xf'@U1_s + Xa'@U2_s
                    zp = [[None] * 2 for _ in range(NT)]
                    for it in range(NT):
                        for dh in range(2):
                            zp[it][dh] = ps_tile(f"zp{s}{it}{dh}")
                    for u, XT in ((0, XfT_s), (1, XaT_s)):
                        for dc in range(8):
                            uin = stpool.tile([128, D], f32, tag="x4k", bufs=2, name="uin")
                            nc.gpsimd.dma_start(
                                uin[:], U_d[2 * s + u][dc * 128 : (dc + 1) * 128, :]
                            )
                            ubf = stpool.tile([128, D], bf16, tag="x2k", bufs=2, name="ubf")
                            nc.vector.tensor_copy(ubf[:], uin[:])
                            for it in range(NT):
                                for dh in range(2):
                                    nc.tensor.matmul(
                                        zp[it][dh][:],
                                        XT[:, dc, it * 128 : (it + 1) * 128],
                                        ubf[:, dh * 512 : (dh + 1) * 512],
                                        start=(u == 0 and dc == 0),
                                        stop=(u == 1 and dc == 7),
                                    )
                    for it in range(NT):
                        for dh in range(2):
                            if s == 0:
                                nc.scalar.activation(
                                    Z_s[:, it, dh * 512 : (dh + 1) * 512],
                                    zp[it][dh][:],
                                    Act.Copy,
                                )
                            else:
                                nc.vector.tensor_tensor(
                                    Z_s[:, it, dh * 512 : (dh + 1) * 512],
                                    Z_s[:, it, dh * 512 : (dh + 1) * 512],
                                    zp[it][dh][:],
                                    Alu.add,
                                )

                # ================= LayerNorm epilogue =================
                if phase < 5:
                    for it in range(NT):
                        dxl = stpool.tile([128, D], f32, tag="x4k", bufs=2, name="dxl")
                        nc.sync.dma_start(dxl[:], Xloc_d[it * 128 : (it + 1) * 128, :])
                        nc.sync.dma_start(out_d[it * 128 : (it + 1) * 128, :], dxl[:])
                ln_on = phase >= 5
                if ln_on:
                    gamma_s1 = spool.tile([1, D], f32, tag="g1", bufs=1, name="gamma_s1")
                    beta_s1 = spool.tile([1, D], f32, tag="b1", bufs=1, name="beta_s1")
                    nc.sync.dma_start(gamma_s1[:], gamma_d[:])
                    nc.sync.dma_start(beta_s1[:], beta_d[:])
                    gamma_bc = bpool.tile([128, D], f32, tag="XfT", bufs=2, name="gamma_bc")
                    beta_bc = bpool.tile([128, D], f32, tag="XfT", bufs=2, name="beta_bc")
                    for dh in range(2):
                        gps = ps_tile(f"gps{dh}")
                        nc.tensor.matmul(
                            gps[:],
                            ones1_s[:],
                            gamma_s1[:, dh * 512 : (dh + 1) * 512],
                            start=True,
                            stop=True,
                        )
                        nc.scalar.activation(
                            gamma_bc[:, dh * 512 : (dh + 1) * 512], gps[:], Act.Copy
                        )
                        bps = ps_tile(f"bps{dh}")
                        nc.tensor.matmul(
                            bps[:],
                            ones1_s[:],
                            beta_s1[:, dh * 512 : (dh + 1) * 512],
                            start=True,
                            stop=True,
                        )
                        nc.scalar.activation(
                            beta_bc[:, dh * 512 : (dh + 1) * 512], bps[:], Act.Copy
                        )

                    inv_d = 1.0 / D
                    for it in range(NT):
                        xlf = stpool.tile([128, D], f32, tag="x4k", bufs=2, name="xlf")
                        nc.sync.dma_start(xlf[:], Xloc_d[it * 128 : (it + 1) * 128, :])
                        Y = bpool.tile([128, D], f32, tag="E", bufs=1, name="Y")
                        nc.vector.tensor_tensor(Y[:], Z_s[:, it, :], xlf[:], Alu.add)
                        scrA = bpool.tile([128, D], f32, tag="Af", bufs=1, name="scrA")
                        sY = spool.tile([128, 1], f32, tag="sY", bufs=2, name="sY")
                        nc.scalar.activation(scrA[:], Y[:], Act.Copy, accum_out=sY[:])
                        scrB = bpool.tile([128, D], f32, tag="ATf", bufs=1, name="scrB")
                        sY2 = spool.tile([128, 1], f32, tag="sY2", bufs=2, name="sY2")
                        nc.scalar.activation(scrB[:], Y[:], Act.Square, accum_out=sY2[:])
                        mu = spool.tile([128, 1], f32, tag="mu", bufs=2, name="mu")
                        nc.vector.tensor_scalar_mul(mu[:], sY[:], inv_d)
                        ex2 = spool.tile([128, 1], f32, tag="ex2", bufs=2, name="ex2")
                        nc.vector.tensor_scalar_mul(ex2[:], sY2[:], inv_d)
                        musq = spool.tile([128, 1], f32, tag="musq", bufs=2, name="musq")
                        nc.vector.tensor_tensor(musq[:], mu[:], mu[:], Alu.mult)
                        var = spool.tile([128, 1], f32, tag="var", bufs=2, name="var")
                        nc.vector.tensor_tensor(var[:], ex2[:], musq[:], Alu.subtract)
                        vpe = spool.tile([128, 1], f32, tag="vpe", bufs=2, name="vpe")
                        nc.vector.tensor_scalar_add(vpe[:], var[:], LN_EPS)
                        sd = spool.tile([128, 1], f32, tag="sd", bufs=2, name="sd")
                        nc.scalar.activation(sd[:], vpe[:], Act.Sqrt)
                        rstd = spool.tile([128, 1], f32, tag="rstd", bufs=2, name="rstd")
                        nc.vector.reciprocal(rstd[:], sd[:])
                        tnorm = bpool.tile([128, D], f32, tag="L", bufs=1, name="tnorm")
                        nc.vector.tensor_scalar(
                            tnorm[:], Y[:], mu[:], rstd[:], Alu.subtract, Alu.mult
                        )
                        nc.vector.tensor_tensor(tnorm[:], tnorm[:], gamma_bc[:], Alu.mult)
                        yout = bpool.tile([128, D], f32, tag="x4k", bufs=2, name="yout")
                        nc.vector.tensor_tensor(yout[:], tnorm[:], beta_bc[:], Alu.add)
                        nc.sync.dma_start(out_d[it * 128 : (it + 1) * 128, :], yout[:])


    nc.finalize()
    return nc


def _get_nc():
    import os

    phase = int(os.environ.get("BASSKPHASE", "5"))
    sub = int(os.environ.get("BASSSUB", "9"))
    iters = int(os.environ.get("BASSITERS", "1"))
    key = ("nc", phase, sub, iters)
    if key not in _CACHE:
        _CACHE[key] = _build(phase, sub, iters)
    return _CACHE[key]


def make_in_maps(inputs):
    X = np.ascontiguousarray(inputs["X"], dtype=np.float32)
    S = np.ascontiguousarray(inputs["S"], dtype=np.float32)
    gamma = np.ascontiguousarray(inputs["gamma"], dtype=np.float32).reshape(1, D)
    beta = np.ascontiguousarray(inputs["beta"], dtype=np.float32).reshape(1, D)
    reps = {
        k: np.ascontiguousarray(inputs[k], dtype=np.float32)
        for k in ("W1", "W2", "W3", "U1_0", "U2_0", "U1_1", "U2_1")
    }
    in_maps = []
    for c in range(NCORES):
        lo, hi = c * NLOC, (c + 1) * NLOC
        m = {
            "X": X,
            "Xloc": np.ascontiguousarray(X[lo:hi]),
            "S": np.ascontiguousarray(S[lo:hi]),
            "gamma": gamma,
            "beta": beta,
        }
        m.update(reps)
        in_maps.append(m)
    return in_maps


def kernel(**inputs):
    from concourse.bass_utils import run_bass_kernel_spmd

    nc = _get_nc()
    in_maps = make_in_maps(inputs)
    res = run_bass_kernel_spmd(nc, in_maps, core_ids=list(range(NCORES)))
    out = np.concatenate([res.results[c]["out"] for c in range(NCORES)], axis=0)
    return np.ascontiguousarray(out, dtype=np.float32)



# revision 7
# speedup vs baseline: 1.0582x; 1.0582x over previous
"""Trainium2 Bass kernel for nn_AdaptiveFunctionBlock (gnn_message_passing).

Row-shards the N=4096 dimension across 8 NeuronCores.  Per core (512 rows):
  step s: L = ((Xa@W1)@W2)@(Xa@W3)^T in f32, per-row top-p (nucleus) selection
  done exactly (segment top-8 candidates -> sorted top-T -> f32 sequential
  cumsum threshold, reference-faithful tie handling via a column correction),
  softmax -> A, then bf16 matmuls  Xa' = A@Xa,  Xf' = recipD*(S@Xf + Xf_loc),
  Z += Xf'@U1_s + Xa'@U2_s,  with an AllGather of (Xf'|Xa') between steps.
  Step-2 logits use associativity: K2 = A1@(X@W3), Q2' = A1@((X@W1)@W2), so the
  f32 work stays O(N^2*64).  Finally LayerNorm(X + Z)*gamma + beta.
"""

import numpy as np

N = 4096
D = 1024
DA = 64
NCORES = 8
NLOC = N // NCORES          # 512
NT = NLOC // 128            # 4 row-tiles per core
NJC = N // 128              # 32 column chunks
P_TOP = 0.9
LN_EPS = 1e-5
T1, T2 = 16, 96             # top-p extraction depth per step
NSEG = 32
SEGW = N // NSEG            # 128
NEGINF = -3.0e38
POSINF = 3.0e38

_CACHE = {}


def _build(phase=5, sub=9, iters=1):
    import concourse.bass as bass
    import concourse.mybir as mybir
    from concourse import bacc, tile

    dt = mybir.dt
    f32 = dt.float32
    bf16 = dt.bfloat16
    i32 = dt.int32
    u32 = dt.uint32
    Act = mybir.ActivationFunctionType
    Alu = mybir.AluOpType
    PSUM = bass.MemorySpace.PSUM
    DRAM = bass.MemorySpace.DRAM

    nc = bacc.Bacc(num_devices=NCORES)

    # ---------------- I/O ----------------
    # host pre-stages: Xb = bf16(X) full, STb = bf16(S_loc^T), recipD =
    # 1/(rowsum+1) as [NLOC,1] f32, U matrices in bf16.
    Xb_d = nc.dram_tensor("Xb", [N, D], bf16, kind="ExternalInput")
    Xloc_d = nc.dram_tensor("Xloc", [NLOC, D], f32, kind="ExternalInput")
    STb_d = nc.dram_tensor("STb", [N, NLOC], bf16, kind="ExternalInput")
    recipD_d = nc.dram_tensor("recipD", [NLOC, 1], f32, kind="ExternalInput")
    W1_d = nc.dram_tensor("W1", [D, DA], f32, kind="ExternalInput")
    W2_d = nc.dram_tensor("W2", [DA, DA], f32, kind="ExternalInput")
    W3_d = nc.dram_tensor("W3", [D, DA], f32, kind="ExternalInput")
    U_d = [
        nc.dram_tensor("U1_0", [D, D], bf16, kind="ExternalInput"),
        nc.dram_tensor("U2_0", [D, D], bf16, kind="ExternalInput"),
        nc.dram_tensor("U1_1", [D, D], bf16, kind="ExternalInput"),
        nc.dram_tensor("U2_1", [D, D], bf16, kind="ExternalInput"),
    ]
    gamma_d = nc.dram_tensor("gamma", [1, D], f32, kind="ExternalInput")
    beta_d = nc.dram_tensor("beta", [1, D], f32, kind="ExternalInput")
    out_d = nc.dram_tensor("out", [NLOC, D], f32, kind="ExternalOutput")

    # ---------------- inline constants ----------------
    ident_c = nc.inline_tensor(np.eye(128, dtype=np.float32), name="ident_c")
    ones1_c = nc.inline_tensor(np.ones((1, 128), dtype=np.float32), name="ones1_c")

    rg = [list(range(NCORES))]
    Tmax = max(T1, T2)

    with tile.TileContext(nc) as tc:
        with (
            tc.tile_pool(name="const", bufs=1) as cpool,
            tc.tile_pool(name="res", bufs=1) as rpool,
            tc.tile_pool(name="big", bufs=1) as bpool,
            tc.tile_pool(name="stream", bufs=1) as stpool,
            tc.tile_pool(name="small", bufs=1) as spool,
            tc.tile_pool(name="psum", bufs=1, space=PSUM) as ppool,
            tc.tile_pool(name="dram", bufs=1, space=DRAM) as dpool,
        ):
            # every PSUM user goes through one tag: 8 x [128,512] banks
            def ps_tile(name):
                return ppool.tile([128, 512], f32, tag="pb", bufs=8, name=name)

            def ps_tile_b(name):
                return ppool.tile([128, 512], bf16, tag="pb", bufs=8, name=name)

            # ---------- SBUF consts ----------
            identf_s = cpool.tile([128, 128], f32, name="identf_s")
            nc.sync.dma_start(identf_s[:], ident_c[:])
            identb_s = cpool.tile([128, 128], bf16, name="identb_s")
            nc.vector.tensor_copy(identb_s[:], identf_s[:])
            ones1_s = cpool.tile([1, 128], f32, name="ones1_s")
            nc.sync.dma_start(ones1_s[:], ones1_c[:])
            zerosT_s = cpool.tile([128, Tmax], f32, name="zerosT_s")
            nc.vector.memset(zerosT_s[:], 0.0)

            # ---------- HBM scratch ----------
            ATb_hbm = dpool.tile([NJC, 128, NLOC], bf16, name="ATb_hbm")

            # ---------- residents ----------
            recipD_s = rpool.tile([128, NT], f32, name="recipD_s")
            Z_s = rpool.tile([128, NT, D], f32, name="Z_s")
            K1P1rm_s = rpool.tile([128, NJC, 128], f32, name="K1P1rm_s")
            Xf_loc = rpool.tile([128, NT, D], bf16, name="Xf_loc")
            Xa_loc = rpool.tile([128, NT, D], bf16, name="Xa_loc")

            for rep_ in range(iters):
                # ================= PROLOGUE =================
                ag_kp1_in = dpool.tile([128, NLOC], f32, name="ag_kp1_in")
                ag_kp1_out = dpool.tile(
                    [NCORES, 128, NLOC], f32, addr_space="Shared", name="ag_kp1_out"
                )
                ag_kp2_in = dpool.tile([128, NLOC], f32, name="ag_kp2_in")
                ag_kp2_out = dpool.tile(
                    [NCORES, 128, NLOC], f32, addr_space="Shared", name="ag_kp2_out"
                )
                ag_x_in = dpool.tile([NLOC, 2 * D], bf16, name="ag_x_in")
                ag_x_out = dpool.tile(
                    [NCORES, NLOC, 2 * D], bf16, addr_space="Shared", name="ag_x_out"
                )
                # Xloc load + bf16 + transpose X^T (for K1/Q1 matmuls)
                Xloc_b = bpool.tile([128, NT, D], bf16, tag="XfT", bufs=2, name="Xloc_b")
                XT_s = bpool.tile([128, 8, NLOC], f32, tag="E", bufs=1, name="XT_s")
                for it in range(NT):
                    xl = stpool.tile([128, D], f32, tag="x4k", bufs=2, name="xl")
                    nc.gpsimd.dma_start(xl[:], Xloc_d[it * 128 : (it + 1) * 128, :])
                    nc.vector.tensor_copy(Xloc_b[:, it, :], xl[:])
                    for dq in range(2):  # 2 groups of 4 transpose blocks
                        tp = ps_tile(f"tpx{it}{dq}")
                        for b in range(4):
                            dc = dq * 4 + b
                            nc.tensor.transpose(
                                tp[:, b * 128 : (b + 1) * 128],
                                xl[:, dc * 128 : (dc + 1) * 128],
                                identf_s[:],
                            )
                        for b in range(4):
                            dc = dq * 4 + b
                            nc.scalar.activation(
                                XT_s[:, dc, it * 128 : (it + 1) * 128],
                                tp[:, b * 128 : (b + 1) * 128],
                                Act.Copy,
                            )

                # W31 = [W3|W1] chunks
                W31_s = stpool.tile([128, 8, 128], f32, tag="x4k", bufs=2, name="W31_s")
                nc.sync.dma_start(
                    W31_s[:, :, 0:DA], W3_d[:].rearrange("(c p) a -> p c a", p=128)
                )
                nc.sync.dma_start(
                    W31_s[:, :, DA:128], W1_d[:].rearrange("(c p) a -> p c a", p=128)
                )
                W2_s = spool.tile([DA, DA], f32, tag="W2", bufs=1, name="W2_s")
                nc.sync.dma_start(W2_s[:], W2_d[:])

                # K1^T|Q1^T = W31^T @ X^T   (f32)
                kqp = ps_tile("kqp")
                for dc in range(8):
                    nc.tensor.matmul(
                        kqp[:],
                        W31_s[:, dc, :],
                        XT_s[:, dc, :],
                        start=(dc == 0),
                        stop=(dc == 7),
                    )
                KP1_s = rpool.tile([128, NLOC], f32, name="KP1_s")
                nc.scalar.activation(KP1_s[0:64, :], kqp[0:64, :], Act.Copy)
                q1t = spool.tile([64, NLOC], f32, tag="q1t", bufs=1, name="q1t")
                nc.scalar.activation(q1t[:], kqp[64:128, :], Act.Copy)
                p1p = ps_tile("p1p")
                nc.tensor.matmul(p1p[0:64, :], W2_s[:], q1t[:], start=True, stop=True)
                nc.scalar.activation(KP1_s[64:128, :], p1p[0:64, :], Act.Copy)

                nc.sync.dma_start(ag_kp1_in[:], KP1_s[:])
                nc.gpsimd.collective_compute(
                    "AllGather",
                    Alu.bypass,
                    replica_groups=rg,
                    ins=[ag_kp1_in[:].opt()],
                    outs=[ag_kp1_out[:].opt()],
                )

                # recipD = 1/(rowsum+1) pre-staged on host
                for it in range(NT):
                    nc.sync.dma_start(
                        recipD_s[:, it : it + 1],
                        recipD_d[it * 128 : (it + 1) * 128, :],
                    )

                # K1^T full  (f32, parts 0:64)
                K1T_s = bpool.tile([128, N // 2], f32, tag="KT", bufs=1, name="K1T_s")
                for r in range(NCORES):
                    ph, off = (0, r) if r < 4 else (64, r - 4)
                    nc.sync.dma_start(
                        K1T_s[ph : ph + 64, off * NLOC : (off + 1) * NLOC],
                        ag_kp1_out[r, 0:64, :],
                    )
                # K1P1 row-major via transposes of AG blocks
                for rq in range(8 if (phase != 1 or sub >= 3) else 0):  # 32 blocks in groups of 4
                    tp3 = ps_tile(f"tpk{rq}")
                    blks = []
                    for b in range(4):
                        g = rq * 4 + b
                        r_, ib = g // NT, g % NT
                        blk = stpool.tile(
                            [128, 128], f32, tag="kpblk", bufs=2, name=f"blk{g}"
                        )
                        nc.sync.dma_start(
                            blk[:], ag_kp1_out[r_, :, ib * 128 : (ib + 1) * 128]
                        )
                        nc.tensor.transpose(
                            tp3[:, b * 128 : (b + 1) * 128], blk[:], identf_s[:]
                        )
                    for b in range(4):
                        g = rq * 4 + b
                        nc.scalar.activation(
                            K1P1rm_s[:, g, :], tp3[:, b * 128 : (b + 1) * 128], Act.Copy
                        )

                KP2_s = rpool.tile([128, NLOC], f32, name="KP2_s")
                kp2ps = None

                # ================= STEPS =================
                nsteps = 0 if phase < 2 else (1 if phase < 4 else 2)
                for s in range(nsteps):
                    T = T1 if s == 0 else T2
                    if s == 0:
                        KPl, KTf = KP1_s, K1T_s
                    else:
                        K2T_s = bpool.tile(
                            [128, N // 2], f32, tag="KT", bufs=1, name="K2T_s"
                        )
                        for r in range(NCORES):
                            ph, off = (0, r) if r < 4 else (r - 4, r - 4)
                            ph = 0 if r < 4 else 64
                            off = r if r < 4 else r - 4
                            nc.sync.dma_start(
                                K2T_s[ph : ph + 64, off * NLOC : (off + 1) * NLOC],
                                ag_kp2_out[r, 0:64, :],
                            )
                        KPl, KTf = KP2_s, K2T_s

                    # P^T duplicated in both partition halves (matmul lhsT must
                    # share base partition with the K^T rhs half it pairs with)
                    PT_s = spool.tile([128, NLOC], f32, tag="PT", bufs=1, name=f"PT{s}")
                    nc.sync.dma_start(PT_s[0:64, :], KPl[64:128, :])
                    nc.sync.dma_start(PT_s[64:128, :], KPl[64:128, :])

                    # per-row-tile: logits -> stats -> A
                    for it in range(NT):
                        L = bpool.tile([128, N], f32, tag="L", bufs=1, name="L")
                        for jc8 in range(8):
                            lp = ps_tile(f"lp{it}{jc8}")
                            kph = 0 if jc8 < 4 else 64
                            kpo = jc8 if jc8 < 4 else jc8 - 4
                            nc.tensor.matmul(
                                lp[:],
                                PT_s[kph : kph + 64, it * 128 : (it + 1) * 128],
                                KTf[kph : kph + 64, kpo * 512 : (kpo + 1) * 512],
                                start=True,
                                stop=True,
                            )
                            nc.scalar.activation(
                                L[:, jc8 * 512 : (jc8 + 1) * 512], lp[:], Act.Copy
                            )

                        if sub < 1:
                            continue
                        # --- candidates: per-segment top-8
                        cand = spool.tile([128, 256], f32, tag="cand", bufs=2, name="cand")
                        for sg in range(NSEG):
                            nc.vector.max(
                                cand[:, sg * 8 : sg * 8 + 8],
                                L[:, sg * SEGW : (sg + 1) * SEGW],
                            )
                        if s == 1:
                            cand_copy = spool.tile(
                                [128, 256], f32, tag="candc", bufs=2, name="cand_copy"
                            )
                            nc.vector.tensor_copy(cand_copy[:], cand[:])

                        if sub < 2:
                            continue
                        # --- extract sorted top-T (destroys cand)
                        V = spool.tile([128, Tmax], f32, tag="V", bufs=2, name="V")
                        for rnd in range(T // 8):
                            nc.vector.max(V[:, rnd * 8 : rnd * 8 + 8], cand[:])
                            if rnd < T // 8 - 1:
                                nc.vector.match_replace(
                                    cand[:], V[:, rnd * 8 : rnd * 8 + 8], cand[:], NEGINF
                                )

                        if sub < 3:
                            continue
                        negm = spool.tile([128, 1], f32, tag="negm", bufs=2, name="negm")
                        nc.vector.tensor_scalar_mul(negm[:], V[:, 0:1], -1.0)

                        E = bpool.tile([128, N], f32, tag="E", bufs=1, name="E")
                        Zrow = spool.tile([128, 1], f32, tag="Zrow", bufs=2, name="Zrow")
                        nc.scalar.activation(
                            E[:], L[:], Act.Exp, bias=negm[:], accum_out=Zrow[:]
                        )
                        EV = spool.tile([128, Tmax], f32, tag="EV", bufs=2, name="EV")
                        nc.scalar.activation(EV[:, 0:T], V[:, 0:T], Act.Exp, bias=negm[:])
                        cs = spool.tile([128, Tmax], f32, tag="cs", bufs=2, name="cs")
                        nc.vector.tensor_tensor_scan(
                            cs[:, 0:T], EV[:, 0:T], zerosT_s[:, 0:T], 0.0, Alu.add, Alu.add
                        )
                        thr = spool.tile([128, 1], f32, tag="thr", bufs=2, name="thr")
                        nc.vector.tensor_scalar_mul(thr[:], Zrow[:], P_TOP)
                        kept = spool.tile([128, Tmax], f32, tag="kept", bufs=2, name="kept")
                        nc.vector.scalar_tensor_tensor(
                            kept[:, 0:T], cs[:, 0:T], thr[:], EV[:, 0:T],
                            Alu.subtract, Alu.is_lt,
                        )
                        scr1 = spool.tile([128, Tmax], f32, tag="scr1", bufs=1, name="scr1")
                        Drow = spool.tile([128, 1], f32, tag="Drow", bufs=2, name="Drow")
                        nc.vector.tensor_tensor(
                            scr1[:, 0:T], EV[:, 0:T], kept[:, 0:T], Alu.mult
                        )
                        nc.vector.tensor_reduce(
                            Drow[:], scr1[:, 0:T], mybir.AxisListType.X, Alu.add
                        )
                        nki = spool.tile([128, Tmax], f32, tag="nki", bufs=1, name="nki")
                        nc.vector.tensor_scalar(
                            nki[:, 0:T], kept[:, 0:T], 0.5, POSINF, Alu.is_lt, Alu.mult
                        )
                        scr2 = spool.tile([128, Tmax], f32, tag="scr2", bufs=1, name="scr2")
                        tau = spool.tile([128, 1], f32, tag="tau", bufs=2, name="tau")
                        nc.vector.tensor_tensor(
                            scr2[:, 0:T], nki[:, 0:T], V[:, 0:T], Alu.add
                        )
                        nc.vector.tensor_reduce(
                            tau[:], scr2[:, 0:T], mybir.AxisListType.X, Alu.min
                        )
                        recD = spool.tile([128, 1], f32, tag="recD", bufs=2, name="recD")
                        nc.vector.reciprocal(recD[:], Drow[:])
                        etau = spool.tile([128, 1], f32, tag="etau", bufs=2, name="etau")
                        nc.scalar.activation(etau[:], tau[:], Act.Exp, bias=negm[:])

                        if s == 1:
                            scrT = spool.tile(
                                [128, Tmax], f32, tag="scrT", bufs=1, name="scrT"
                            )
                            rr = spool.tile([128, 1], f32, tag="rr", bufs=2, name="rr")
                            nc.vector.scalar_tensor_tensor(
                                scrT[:, 0:T], V[:, 0:T], tau[:], kept[:, 0:T],
                                Alu.is_equal, Alu.mult,
                            )
                            nc.vector.tensor_reduce(
                                rr[:], scrT[:, 0:T], mybir.AxisListType.X, Alu.add
                            )
                            scr256 = spool.tile(
                                [128, 256], f32, tag="scr256", bufs=1, name="scr256"
                            )
                            ceq = spool.tile([128, 1], f32, tag="ceq", bufs=2, name="ceq")
                            nc.vector.tensor_scalar(
                                scr256[:], cand_copy[:], tau[:], None, Alu.is_equal
                            )
                            nc.vector.tensor_reduce(
                                ceq[:], scr256[:], mybir.AxisListType.X, Alu.add
                            )
                            # w2 = (ceq - r)/ceq * etau * recD  (spread over all ties)
                            wv = spool.tile([128, 1], f32, tag="wv", bufs=2, name="wv")
                            nc.vector.tensor_tensor(wv[:], ceq[:], rr[:], Alu.subtract)
                            nc.vector.tensor_tensor(wv[:], wv[:], etau[:], Alu.mult)
                            nc.vector.tensor_tensor(wv[:], wv[:], recD[:], Alu.mult)
                            rceq = spool.tile([128, 1], f32, tag="rceq", bufs=2, name="rceq")
                            nc.vector.reciprocal(rceq[:], ceq[:])
                            nc.vector.tensor_tensor(wv[:], wv[:], rceq[:], Alu.mult)

                        if sub < 4:
                            continue
                        # --- A materialization (E in-place): E := (E >= etau) * E
                        nc.vector.scalar_tensor_tensor(
                            E[:], E[:], etau[:], E[:], Alu.is_ge, Alu.mult
                        )
                        if s == 0:
                            Af = bpool.tile([128, N], f32, tag="Af", bufs=1, name="Af")
                            nc.vector.tensor_scalar_mul(Af[:], E[:], recD[:])
                            ATf = bpool.tile(
                                [128, NJC, 128], f32, tag="ATf", bufs=1, name="ATf"
                            )
                            for jq in range(8):
                                tp4 = ps_tile(f"tpa{it}{jq}")
                                for b in range(4):
                                    jc = jq * 4 + b
                                    nc.tensor.transpose(
                                        tp4[:, b * 128 : (b + 1) * 128],
                                        Af[:, jc * 128 : (jc + 1) * 128],
                                        identf_s[:],
                                    )
                                for b in range(4):
                                    jc = jq * 4 + b
                                    nc.scalar.activation(
                                        ATf[:, jc, :],
                                        tp4[:, b * 128 : (b + 1) * 128],
                                        Act.Copy,
                                    )
                            ATb = stpool.tile(
                                [128, NJC, 128], bf16, tag="ATb", bufs=1, name="ATb"
                            )
                            nc.vector.tensor_copy(ATb[:], ATf[:])
                            nc.sync.dma_start(
                                ATb_hbm[:, :, it * 128 : (it + 1) * 128].rearrange(
                                    "c p i -> p c i"
                                ),
                                ATb[:],
                            )
                            # K2P2^T accumulation (held psum bank across it loop)
                            if it == 0:
                                kp2ps = ps_tile("kp2ps")
                            for jc in range(NJC):
                                nc.tensor.matmul(
                                    kp2ps[:, it * 128 : (it + 1) * 128],
                                    K1P1rm_s[:, jc, :],
                                    ATf[:, jc, :],
                                    start=(jc == 0),
                                    stop=(jc == NJC - 1),
                                )
                        else:
                            Ab = bpool.tile([128, N], bf16, tag="Af", bufs=1, name="Ab")
                            nc.vector.tensor_scalar_mul(Ab[:], E[:], recD[:])
                            # tie-drop correction: A[:, L==tau] -= w2 (rows identical)
                            eqw = bpool.tile([128, N], f32, tag="ATf", bufs=1, name="eqw")
                            nc.vector.tensor_scalar(
                                eqw[:], L[:], tau[:], wv[:], Alu.is_equal, Alu.mult
                            )
                            nc.vector.tensor_tensor(Ab[:], Ab[:], eqw[:], Alu.subtract)
                            ATb2 = stpool.tile(
                                [128, NJC, 128], bf16, tag="ATb", bufs=1, name="ATb2"
                            )
                            for jq in range(8):
                                tp5 = ps_tile_b(f"tpb{it}{jq}")
                                for b in range(4):
                                    jc = jq * 4 + b
                                    nc.tensor.transpose(
                                        tp5[:, b * 128 : (b + 1) * 128],
                                        Ab[:, jc * 128 : (jc + 1) * 128],
                                        identb_s[:],
                                    )
                                for b in range(4):
                                    jc = jq * 4 + b
                                    nc.scalar.activation(
                                        ATb2[:, jc, :],
                                        tp5[:, b * 128 : (b + 1) * 128],
                                        Act.Copy,
                                    )
                            nc.sync.dma_start(
                                ATb_hbm[:, :, it * 128 : (it + 1) * 128].rearrange(
                                    "c p i -> p c i"
                                ),
                                ATb2[:],
                            )

                    if s == 0 and kp2ps is not None:
                        nc.scalar.activation(KP2_s[:], kp2ps[:], Act.Copy)
                        nc.sync.dma_start(ag_kp2_in[:], KP2_s[:])
                        nc.gpsimd.collective_compute(
                            "AllGather",
                            Alu.bypass,
                            replica_groups=rg,
                            ins=[ag_kp2_in[:].opt()],
                            outs=[ag_kp2_out[:].opt()],
                        )

                    if phase < 3:
                        continue

                    # ---------- big matmuls
                    xdiag = Xloc_b if s == 0 else Xf_loc
                    for dh in range(2):
                        xfp = [None] * NT
                        xap = [None] * NT
                        for it in range(NT):
                            xfp[it] = ps_tile(f"xfp{s}{dh}{it}")
                            xap[it] = ps_tile(f"xap{s}{dh}{it}")
                        for jc in range(NJC):
                            stch = stpool.tile(
                                [128, NLOC], bf16, tag="stch", bufs=2, name="stch"
                            )
                            nc.sync.dma_start(
                                stch[:], STb_d[jc * 128 : (jc + 1) * 128, :]
                            )
                            atch = stpool.tile(
                                [128, NLOC], bf16, tag="atch", bufs=2, name="atch"
                            )
                            nc.sync.dma_start(atch[:], ATb_hbm[jc, :, :])
                            if s == 0:
                                xch_f = stpool.tile(
                                    [128, 512], bf16, tag="xch", bufs=2, name="xchf"
                                )
                                nc.sync.dma_start(
                                    xch_f[:],
                                    Xb_d[jc * 128 : (jc + 1) * 128, dh * 512 : (dh + 1) * 512],
                                )
                                xch_a = xch_f
                            else:
                                r_, blk_ = jc // NT, jc % NT
                                xch_f = stpool.tile(
                                    [128, 512], bf16, tag="xch", bufs=2, name="xchf"
                                )
                                nc.sync.dma_start(
                                    xch_f[:],
                                    ag_x_out[
                                        r_,
                                        blk_ * 128 : (blk_ + 1) * 128,
                                        dh * 512 : (dh + 1) * 512,
                                    ],
                                )
                                xch_a = stpool.tile(
                                    [128, 512], bf16, tag="xcha", bufs=2, name="xcha"
                                )
                                nc.sync.dma_start(
                                    xch_a[:],
                                    ag_x_out[
                                        r_,
                                        blk_ * 128 : (blk_ + 1) * 128,
                                        D + dh * 512 : D + (dh + 1) * 512,
                                    ],
                                )
                            for it in range(NT):
                                nc.tensor.matmul(
                                    xfp[it][:],
                                    stch[:, it * 128 : (it + 1) * 128],
                                    xch_f[:],
                                    start=(jc == 0),
                                    stop=(jc == NJC - 1),
                                )
                                nc.tensor.matmul(
                                    xap[it][:],
                                    atch[:, it * 128 : (it + 1) * 128],
                                    xch_a[:],
                                    start=(jc == 0),
                                    stop=(jc == NJC - 1),
                                )
                        for it in range(NT):
                            # Xf' = recipD*(psum + xdiag)
                            tsum = stpool.tile(
                                [128, 512], f32, tag="tsum", bufs=1, name="tsum"
                            )
                            nc.vector.tensor_tensor(
                                tsum[:],
                                xfp[it][:],
                                xdiag[:, it, dh * 512 : (dh + 1) * 512],
                                Alu.add,
                            )
                            nc.vector.tensor_scalar_mul(
                                Xf_loc[:, it, dh * 512 : (dh + 1) * 512],
                                tsum[:],
                                recipD_s[:, it : it + 1],
                            )
                            nc.vector.tensor_copy(
                                Xa_loc[:, it, dh * 512 : (dh + 1) * 512], xap[it][:]
                            )

                    if s == 0:
                        for it in range(NT):
                            nc.sync.dma_start(
                                ag_x_in[it * 128 : (it + 1) * 128, 0:D], Xf_loc[:, it, :]
                            )
                            nc.sync.dma_start(
                                ag_x_in[it * 128 : (it + 1) * 128, D : 2 * D],
                                Xa_loc[:, it, :],
                            )
                        nc.gpsimd.collective_compute(
                            "AllGather",
                            Alu.bypass,
                            replica_groups=rg,
                            ins=[ag_x_in[:].opt()],
                            outs=[ag_x_out[:].opt()],
                        )

                    # ---------- Xf^T / Xa^T
                    XfT_s = bpool.tile(
                        [128, 8, NLOC], bf16, tag="XfT", bufs=2, name=f"XfT{s}"
                    )
                    XaT_s = bpool.tile(
                        [128, 8, NLOC], bf16, tag="XfT", bufs=2, name=f"XaT{s}"
                    )
                    for src, dst in ((Xf_loc, XfT_s), (Xa_loc, XaT_s)):
                        for dc in range(8):
                            tpt = ps_tile_b(f"tpt{s}{dc}")
                            for it in range(NT):
                                nc.tensor.transpose(
                                    tpt[:, it * 128 : (it + 1) * 128],
                                    src[:, it, dc * 128 : (dc + 1) * 128],
                                    identb_s[:],
                                )
                            nc.scalar.activation(dst[:, dc, :], tpt[:], Act.Copy)

                    # ---------- U products: Z += Xf'@U1_s + Xa'@U2_s
                    zp = [[None] * 2 for _ in range(NT)]
                    for it in range(NT):
                        for dh in range(2):
                            zp[it][dh] = ps_tile(f"zp{s}{it}{dh}")
                    for u, XT in ((0, XfT_s), (1, XaT_s)):
                        for dc in range(8):
                            ubf = stpool.tile([128, D], bf16, tag="x2k", bufs=2, name="ubf")
                            nc.gpsimd.dma_start(
                                ubf[:], U_d[2 * s + u][dc * 128 : (dc + 1) * 128, :]
                            )
                            for it in range(NT):
                                for dh in range(2):
                                    nc.tensor.matmul(
                                        zp[it][dh][:],
                                        XT[:, dc, it * 128 : (it + 1) * 128],
                                        ubf[:, dh * 512 : (dh + 1) * 512],
                                        start=(u == 0 and dc == 0),
                                        stop=(u == 1 and dc == 7),
                                    )
                    for it in range(NT):
                        for dh in range(2):
                            if s == 0:
                                nc.scalar.activation(
                                    Z_s[:, it, dh * 512 : (dh + 1) * 512],
                                    zp[it][dh][:],
                                    Act.Copy,
                                )
                            else:
                                nc.vector.tensor_tensor(
                                    Z_s[:, it, dh * 512 : (dh + 1) * 512],
                                    Z_s[:, it, dh * 512 : (dh + 1) * 512],
                                    zp[it][dh][:],
                                    Alu.add,
                                )

                # ================= LayerNorm epilogue =================
                if phase < 5:
                    for it in range(NT):
                        dxl = stpool.tile([128, D], f32, tag="x4k", bufs=2, name="dxl")
                        nc.sync.dma_start(dxl[:], Xloc_d[it * 128 : (it + 1) * 128, :])
                        nc.sync.dma_start(out_d[it * 128 : (it + 1) * 128, :], dxl[:])
                ln_on = phase >= 5
                if ln_on:
                    gamma_s1 = spool.tile([1, D], f32, tag="g1", bufs=1, name="gamma_s1")
                    beta_s1 = spool.tile([1, D], f32, tag="b1", bufs=1, name="beta_s1")
                    nc.sync.dma_start(gamma_s1[:], gamma_d[:])
                    nc.sync.dma_start(beta_s1[:], beta_d[:])
                    gamma_bc = bpool.tile([128, D], f32, tag="XfT", bufs=2, name="gamma_bc")
                    beta_bc = bpool.tile([128, D], f32, tag="XfT", bufs=2, name="beta_bc")
                    for dh in range(2):
                        gps = ps_tile(f"gps{dh}")
                        nc.tensor.matmul(
                            gps[:],
                            ones1_s[:],
                            gamma_s1[:, dh * 512 : (dh + 1) * 512],
                            start=True,
                            stop=True,
                        )
                        nc.scalar.activation(
                            gamma_bc[:, dh * 512 : (dh + 1) * 512], gps[:], Act.Copy
                        )
                        bps = ps_tile(f"bps{dh}")
                        nc.tensor.matmul(
                            bps[:],
                            ones1_s[:],
                            beta_s1[:, dh * 512 : (dh + 1) * 512],
                            start=True,
                            stop=True,
                        )
                        nc.scalar.activation(
                            beta_bc[:, dh * 512 : (dh + 1) * 512], bps[:], Act.Copy
                        )

                    inv_d = 1.0 / D
                    for it in range(NT):
                        xlf = stpool.tile([128, D], f32, tag="x4k", bufs=2, name="xlf")
                        nc.sync.dma_start(xlf[:], Xloc_d[it * 128 : (it + 1) * 128, :])
                        Y = bpool.tile([128, D], f32, tag="E", bufs=1, name="Y")
                        nc.vector.tensor_tensor(Y[:], Z_s[:, it, :], xlf[:], Alu.add)
                        scrA = bpool.tile([128, D], f32, tag="Af", bufs=1, name="scrA")
                        sY = spool.tile([128, 1], f32, tag="sY", bufs=2, name="sY")
                        nc.scalar.activation(scrA[:], Y[:], Act.Copy, accum_out=sY[:])
                        scrB = bpool.tile([128, D], f32, tag="ATf", bufs=1, name="scrB")
                        sY2 = spool.tile([128, 1], f32, tag="sY2", bufs=2, name="sY2")
                        nc.scalar.activation(scrB[:], Y[:], Act.Square, accum_out=sY2[:])
                        mu = spool.tile([128, 1], f32, tag="mu", bufs=2, name="mu")
                        nc.vector.tensor_scalar_mul(mu[:], sY[:], inv_d)
                        ex2 = spool.tile([128, 1], f32, tag="ex2", bufs=2, name="ex2")
                        nc.vector.tensor_scalar_mul(ex2[:], sY2[:], inv_d)
                        musq = spool.tile([128, 1], f32, tag="musq", bufs=2, name="musq")
                        nc.vector.tensor_tensor(musq[:], mu[:], mu[:], Alu.mult)
                        var = spool.tile([128, 1], f32, tag="var", bufs=2, name="var")
                        nc.vector.tensor_tensor(var[:], ex2[:], musq[:], Alu.subtract)
                        vpe = spool.tile([128, 1], f32, tag="vpe", bufs=2, name="vpe")
                        nc.vector.tensor_scalar_add(vpe[:], var[:], LN_EPS)
                        sd = spool.tile([128, 1], f32, tag="sd", bufs=2, name="sd")
                        nc.scalar.activation(sd[:], vpe[:], Act.Sqrt)
                        rstd = spool.tile([128, 1], f32, tag="rstd", bufs=2, name="rstd")
                        nc.vector.reciprocal(rstd[:], sd[:])
                        tnorm = bpool.tile([128, D], f32, tag="L", bufs=1, name="tnorm")
                        nc.vector.tensor_scalar(
                            tnorm[:], Y[:], mu[:], rstd[:], Alu.subtract, Alu.mult
                        )
                        nc.vector.tensor_tensor(tnorm[:], tnorm[:], gamma_bc[:], Alu.mult)
                        yout = bpool.tile([128, D], f32, tag="x4k", bufs=2, name="yout")
                        nc.vector.tensor_tensor(yout[:], tnorm[:], beta_bc[:], Alu.add)
                        nc.sync.dma_start(out_d[it * 128 : (it + 1) * 128, :], yout[:])


    nc.finalize()
    return nc


def _get_nc():
    import os

    phase = int(os.environ.get("BASSKPHASE", "5"))
    sub = int(os.environ.get("BASSSUB", "9"))
    iters = int(os.environ.get("BASSITERS", "1"))
    key = ("nc", phase, sub, iters)
    if key not in _CACHE:
        _CACHE[key] = _build(phase, sub, iters)
    return _CACHE[key]


def make_in_maps(inputs):
    import ml_dtypes

    bf16 = ml_dtypes.bfloat16
    X = np.ascontiguousarray(inputs["X"], dtype=np.float32)
    S = np.ascontiguousarray(inputs["S"], dtype=np.float32)
    Xb = X.astype(bf16)
    gamma = np.ascontiguousarray(inputs["gamma"], dtype=np.float32).reshape(1, D)
    beta = np.ascontiguousarray(inputs["beta"], dtype=np.float32).reshape(1, D)
    reps = {
        k: np.ascontiguousarray(inputs[k], dtype=np.float32)
        for k in ("W1", "W2", "W3")
    }
    reps.update(
        {
            k: np.ascontiguousarray(inputs[k]).astype(bf16)
            for k in ("U1_0", "U2_0", "U1_1", "U2_1")
        }
    )
    in_maps = []
    for c in range(NCORES):
        lo, hi = c * NLOC, (c + 1) * NLOC
        Sl = S[lo:hi]
        m = {
            "Xb": Xb,
            "Xloc": np.ascontiguousarray(X[lo:hi]),
            "STb": np.ascontiguousarray(Sl.T).astype(bf16),
            "recipD": (1.0 / (Sl.sum(axis=1) + 1.0)).astype(np.float32).reshape(
                NLOC, 1
            ),
            "gamma": gamma,
            "beta": beta,
        }
        m.update(reps)
        in_maps.append(m)
    return in_maps


def kernel(**inputs):
    from concourse.bass_utils import run_bass_kernel_spmd

    nc = _get_nc()
    in_maps = make_in_maps(inputs)
    res = run_bass_kernel_spmd(nc, in_maps, core_ids=list(range(NCORES)))
    out = np.concatenate([res.results[c]["out"] for c in range(NCORES)], axis=0)
    return np.ascontiguousarray(out, dtype=np.float32)

